# revision 1
# baseline (speedup 1.0000x reference)
"""Trainium2 Bass kernel for the 2-layer GAT network (nn_CGAT).

Self-contained: hardcodes all shapes. Strategy:
  * Algebraic restructuring: aggregate raw features first, apply folded weight
    products after (W1->conv1 and W2->conv2 folded on host, weights only).
  * Max-free softmax (logits are bounded ~ +-6, validated): exp(l)/sum(exp(l)).
  * Layer 1 sharded by destination tiles (dst-sorted edges, greedy-packed tiles
    of <=128 dsts / <=1280 edges = 10 blocks of 128 edges).
  * Layer 2 sharded by source (owner of h1 row); per-dst2 partial sums + final
    ReduceScatter of partial (agg2 @ M2 | denom2).
  * Per-core attention-logit tables built data-parallel and AllGathered.
"""
import math
import numpy as np
import ml_dtypes

import concourse.bass as bass
from concourse import bacc
import concourse.tile as tile
from concourse import mybir
from concourse.bass_utils import run_bass_kernel_spmd

P = 128
C = 8                      # cores
N0, N1, N2 = 80000, 40000, 8000
E1, E2 = 400000, 128000
F, HID, OUT, H = 128, 256, 128, 4
NEG = 0.2
CAP1, BLK1 = 1280, 10      # L1 tile: <=128 dsts, 10 blocks of 128 edges
CAP2, BLK2 = 256, 2        # L2 tile: <=128 dsts, 2 blocks of 128 edges
NSL = 10000                # dense-phase node rows per core
NSLP = 10112               # padded to x128
BF16 = ml_dtypes.bfloat16


# ---------------------------------------------------------------- host side --
def _greedy_tiles(deg, cap, max_d=P):
    """Pack consecutive dsts into tiles with <= cap edges and <= max_d dsts.
    Every dst appears in exactly one tile (zero-degree dsts included).
    Returns list of (d0, nd, e0, ne)."""
    tiles = []
    n = len(deg)
    d = 0
    e = 0
    while d < n:
        d0, e0 = d, e
        ne = 0
        nd = 0
        while d < n and nd < max_d and ne + deg[d] <= cap:
            ne += int(deg[d]); d += 1; nd += 1
        assert nd > 0, "single dst exceeds tile capacity"
        tiles.append((d0, nd, e0, ne))
        e += ne
    return tiles


def preprocess(inputs):
    x = np.ascontiguousarray(np.asarray(inputs["x"], np.float32))
    s1 = np.asarray(inputs["edge_src1"]).astype(np.int64)
    d1 = np.asarray(inputs["edge_dst1"]).astype(np.int64)
    s2 = np.asarray(inputs["edge_src2"]).astype(np.int64)
    d2 = np.asarray(inputs["edge_dst2"]).astype(np.int64)
    W1 = np.asarray(inputs["W1"], np.float32)
    att_s1 = np.asarray(inputs["att_src1"], np.float32)
    att_d1 = np.asarray(inputs["att_dst1"], np.float32)
    b1 = np.asarray(inputs["b1"], np.float32)
    W2 = np.asarray(inputs["W2"], np.float32)
    att_s2 = np.asarray(inputs["att_src2"], np.float32)
    att_d2 = np.asarray(inputs["att_dst2"], np.float32)
    b2 = np.asarray(inputs["b2"], np.float32)
    c1w = np.asarray(inputs["conv1_w"], np.float32)
    c1b = np.asarray(inputs["conv1_b"], np.float32)
    c2w = np.asarray(inputs["conv2_w"], np.float32)
    c2b = np.asarray(inputs["conv2_b"], np.float32)

    # folded weights (tiny, fp32 on host)
    W1h = W1.reshape(F, H, F)
    Wa_sd1 = np.concatenate([np.einsum('fhc,hc->fh', W1h, att_s1),
                             np.einsum('fhc,hc->fh', W1h, att_d1)], axis=1)  # [128, 8]
    c1wT = c1w.T
    M1 = np.stack([W1h[:, h, :] @ c1wT[h*F:(h+1)*F] for h in range(H)])  # [4,128,256]
    cc1 = c1b + c1w @ b1                                                  # [256]
    Wa2 = np.stack([W2 @ att_s2[0], W2 @ att_d2[0]], axis=1)              # [256, 2]
    M2 = W2 @ c2w.T                                                       # [256,128]
    cc2 = c2b + c2w @ b2                                                  # [128]
    c1W2 = cc1 @ Wa2                                                      # [2]

    # node -> table1 row (dense slices of 10000 padded to 10112 per core)
    def t1row(n):
        return NSLP * (n // NSL) + (n % NSL)

    # ---- L1 tiling ----
    deg1 = np.bincount(d1, minlength=N1)
    order1 = np.argsort(d1, kind='stable')
    s1s, d1s = s1[order1], d1[order1]
    tiles1 = _greedy_tiles(deg1, CAP1)
    T1 = len(tiles1)
    NT1 = math.ceil(T1 / C)
    # node ownership for n in [0, N1)
    own = np.empty(N1, np.int32); tloc = np.empty(N1, np.int32); dloc = np.empty(N1, np.int32)
    for i, (d0, nd, e0, ne) in enumerate(tiles1):
        own[d0:d0+nd] = i // NT1
        tloc[d0:d0+nd] = i % NT1
        dloc[d0:d0+nd] = np.arange(nd)
    h1row = (NT1 * P) * own.astype(np.int64) + P * tloc.astype(np.int64) + dloc.astype(np.int64)

    NB1 = NT1 * BLK1
    m_src1 = np.zeros((C, P, NB1), np.int32)
    m_xsrc1 = np.zeros((C, P, NB1), np.int32)
    m_dstloc1 = np.full((C, P, NB1), float(P), np.float32)
    m_start1 = np.zeros((C, P, NT1), np.float32)
    m_end1 = np.zeros((C, P, NT1), np.float32)
    m_adidx1 = np.zeros((C, P, NT1), np.int32)
    for i, (d0, nd, e0, ne) in enumerate(tiles1):
        c, t = i // NT1, i % NT1
        esrc = s1s[e0:e0+ne]
        edl = (d1s[e0:e0+ne] - d0)
        pad = BLK1 * P - ne
        col = np.full(BLK1 * P, 0, np.int64)
        col[:ne] = t1row(esrc)
        xcol = np.full(BLK1 * P, 0, np.int64)
        xcol[:ne] = esrc
        dlc = np.full(BLK1 * P, float(P), np.float32)
        dlc[:ne] = edl.astype(np.float32)
        m_src1[c, :, t*BLK1:(t+1)*BLK1] = col.reshape(BLK1, P).T
        m_xsrc1[c, :, t*BLK1:(t+1)*BLK1] = xcol.reshape(BLK1, P).T
        m_dstloc1[c, :, t*BLK1:(t+1)*BLK1] = dlc.reshape(BLK1, P).T
        cum = np.zeros(nd + 1, np.int64)
        cum[1:] = np.cumsum(deg1[d0:d0+nd])
        m_start1[c, :nd, t] = cum[:-1]
        m_end1[c, :nd, t] = cum[1:]
        m_adidx1[c, :nd, t] = t1row(np.arange(d0, d0+nd))

    # ---- L2 ----
    own2 = own[s2]                     # owner core of each layer-2 edge
    deg_all2 = np.bincount(d2, minlength=N2)
    maxdeg2 = int(deg_all2.max())
    assert maxdeg2 <= CAP2
    NT2 = 0
    pc_edges = []
    for c in range(C):
        sel = own2 == c
        es, ed = s2[sel], d2[sel]
        o = np.argsort(ed, kind='stable')
        es, ed = es[o], ed[o]
        degc = np.bincount(ed, minlength=N2)
        tl = _greedy_tiles(degc, CAP2)
        pc_edges.append((es, ed, degc, tl))
        NT2 = max(NT2, len(tl))
    NB2 = NT2 * BLK2
    m_src2 = np.zeros((C, P, NB2), np.int32)
    m_a2idx = np.zeros((C, P, NB2), np.int32)
    m_dstloc2 = np.full((C, P, NB2), float(P), np.float32)
    m_start2 = np.zeros((C, P, NT2), np.float32)
    m_end2 = np.zeros((C, P, NT2), np.float32)
    m_ad2idx = np.zeros((C, P, NT2), np.int32)
    m_o2idx = np.full((C, P, NT2), N2, np.int32)
    for c in range(C):
        es, ed, degc, tl = pc_edges[c]
        for t, (d0, nd, e0, ne) in enumerate(tl):
            # local h1_slice row of src node (owner == c by construction)
            col = np.zeros(BLK2 * P, np.int64)
            col[:ne] = P * tloc[es[e0:e0+ne]] + dloc[es[e0:e0+ne]]
            a2c = np.zeros(BLK2 * P, np.int64)
            a2c[:ne] = h1row[es[e0:e0+ne]]
            dlc = np.full(BLK2 * P, float(P), np.float32)
            dlc[:ne] = (ed[e0:e0+ne] - d0).astype(np.float32)
            m_src2[c, :, t*BLK2:(t+1)*BLK2] = col.reshape(BLK2, P).T
            m_a2idx[c, :, t*BLK2:(t+1)*BLK2] = a2c.reshape(BLK2, P).T
            m_dstloc2[c, :, t*BLK2:(t+1)*BLK2] = dlc.reshape(BLK2, P).T
            cum = np.zeros(nd + 1, np.int64)
            cum[1:] = np.cumsum(degc[d0:d0+nd])
            m_start2[c, :nd, t] = cum[:-1]
            m_end2[c, :nd, t] = cum[1:]
            m_ad2idx[c, :nd, t] = h1row[np.arange(d0, d0+nd)]
            m_o2idx[c, :nd, t] = np.arange(d0, d0+nd)

    # constants shipped to every core
    iota_row = np.tile(np.arange(P, dtype=np.float32), (P, 1)).astype(BF16)
    iota_e = np.tile(np.arange(CAP1, dtype=np.float32), (P, 1))
    ident = np.eye(P, dtype=np.float32)
    M1_sb = np.concatenate([M1[h] for h in range(H)], axis=1).astype(BF16)   # [128, 4*256]
    M1W2 = np.concatenate([M1[h] @ Wa2 for h in range(H)], axis=1).astype(BF16)  # [128, 4*2]
    M2_sb = np.concatenate([M2[0:P, :], M2[P:2*P, :]], axis=1).astype(BF16)  # [128, 2*128]
    c1_b = np.tile(cc1[None, :], (P, 1)).astype(np.float32)                  # [128,256]
    c1w2_b = np.tile(c1W2[None, :], (P, 1)).astype(np.float32)               # [128,2]
    cc2_b = np.tile(cc2[None, :], (P, 1)).astype(np.float32)                 # [128,128]

    in_maps = []
    for c in range(C):
        xs = np.zeros((NSLP, F), np.float32)
        xs[:NSL] = x[c*NSL:(c+1)*NSL]
        in_maps.append(dict(
            x=x, x_slice=xs,
            m_xsrc1=m_xsrc1[c], m_dstloc1=m_dstloc1[c],
            m_start1=m_start1[c], m_end1=m_end1[c], m_adidx1=m_adidx1[c],
            m_src2=m_src2[c], m_dstloc2=m_dstloc2[c],
            m_start2=m_start2[c], m_end2=m_end2[c], m_ad2idx=m_ad2idx[c],
            m_o2idx=m_o2idx[c],
            wa_sd1=Wa_sd1, m1=M1_sb, m1w2=M1W2, m2=M2_sb,
            c1_b=c1_b, c1w2_b=c1w2_b, cc2_b=cc2_b,
            iota_row=iota_row, iota_e=iota_e, ident=ident,
        ))
    plan = dict(NT1=NT1, NT2=NT2)
    return in_maps, plan


# -------------------------------------------------------------- kernel build --
def build_kernel(NT1, NT2, DEBUG=False):
    nc = bacc.Bacc("TRN2", target_bir_lowering=False, debug=False, num_devices=C)
    dt = mybir.dt
    NB1, NB2 = NT1 * BLK1, NT2 * BLK2
    HSL = NT1 * P + 8          # h1 slice rows (tile-major) + pad
    T2R = C * NT1 * P          # table2 rows

    # inputs
    x = nc.declare_dram_parameter("x", [N0, F], dt.float32, isOutput=False)
    x_slice = nc.declare_dram_parameter("x_slice", [NSLP, F], dt.float32, isOutput=False)
    m_xsrc1 = nc.declare_dram_parameter("m_xsrc1", [P, NB1], dt.int32, isOutput=False)
    m_dstloc1 = nc.declare_dram_parameter("m_dstloc1", [P, NB1], dt.float32, isOutput=False)
    m_start1 = nc.declare_dram_parameter("m_start1", [P, NT1], dt.float32, isOutput=False)
    m_end1 = nc.declare_dram_parameter("m_end1", [P, NT1], dt.float32, isOutput=False)
    m_adidx1 = nc.declare_dram_parameter("m_adidx1", [P, NT1], dt.int32, isOutput=False)
    m_src2 = nc.declare_dram_parameter("m_src2", [P, NB2], dt.int32, isOutput=False)
    m_dstloc2 = nc.declare_dram_parameter("m_dstloc2", [P, NB2], dt.float32, isOutput=False)
    m_start2 = nc.declare_dram_parameter("m_start2", [P, NT2], dt.float32, isOutput=False)
    m_end2 = nc.declare_dram_parameter("m_end2", [P, NT2], dt.float32, isOutput=False)
    m_ad2idx = nc.declare_dram_parameter("m_ad2idx", [P, NT2], dt.int32, isOutput=False)
    m_o2idx = nc.declare_dram_parameter("m_o2idx", [P, NT2], dt.int32, isOutput=False)
    wa_sd1 = nc.declare_dram_parameter("wa_sd1", [F, 2*H], dt.float32, isOutput=False)
    m1 = nc.declare_dram_parameter("m1", [F, H*HID], dt.bfloat16, isOutput=False)
    m1w2 = nc.declare_dram_parameter("m1w2", [F, H*2], dt.bfloat16, isOutput=False)
    m2 = nc.declare_dram_parameter("m2", [P, 2*OUT], dt.bfloat16, isOutput=False)
    c1_b = nc.declare_dram_parameter("c1_b", [P, HID], dt.float32, isOutput=False)
    c1w2_b = nc.declare_dram_parameter("c1w2_b", [P, 2], dt.float32, isOutput=False)
    cc2_b = nc.declare_dram_parameter("cc2_b", [P, OUT], dt.float32, isOutput=False)
    iota_row = nc.declare_dram_parameter("iota_row", [P, P], dt.bfloat16, isOutput=False)
    iota_e = nc.declare_dram_parameter("iota_e", [P, CAP1], dt.float32, isOutput=False)
    ident = nc.declare_dram_parameter("ident", [P, P], dt.float32, isOutput=False)

    out_ext = nc.declare_dram_parameter("out", [N2 // C, OUT], dt.float32, isOutput=True)
    if DEBUG:
        dbg_selT = nc.declare_dram_parameter("dbg_selT", [P, CAP1], dt.bfloat16, isOutput=True)
        dbg_sel = nc.declare_dram_parameter("dbg_sel", [P, BLK1 * P], dt.bfloat16, isOutput=True)
        dbg_l = nc.declare_dram_parameter("dbg_l", [P, BLK1 * H], dt.float32, isOutput=True)
        dbg_p = nc.declare_dram_parameter("dbg_p", [P, BLK1 * H], dt.bfloat16, isOutput=True)
        dbg_rec = nc.declare_dram_parameter("dbg_rec", [P, H], dt.bfloat16, isOutput=True)
        dbg_al = nc.declare_dram_parameter("dbg_al", [P, BLK1 * H], dt.float32, isOutput=True)
        dbg_aggT = nc.declare_dram_parameter("dbg_aggT", [P, H * P], dt.bfloat16, isOutput=True)
        dbg_h1 = nc.declare_dram_parameter("dbg_h1", [P, HID], dt.float32, isOutput=True)
        dbg_a2 = nc.declare_dram_parameter("dbg_a2", [P, 2], dt.float32, isOutput=True)
        dbg_asg = nc.declare_dram_parameter("dbg_asg", [P, BLK1 * 2*H], dt.float32, isOutput=True)
        dbg_adt = nc.declare_dram_parameter("dbg_adt", [P, 2*H], dt.float32, isOutput=True)
        dbg_xar = nc.declare_dram_parameter("dbg_xar", [P, BLK1 * F], dt.bfloat16, isOutput=True)

    # internal DRAM
    asd_slice = nc.dram_tensor("asd_slice", [NSLP, 2*H], dt.float32)
    table1 = nc.dram_tensor("table1", [C * NSLP, 2*H], dt.float32, addr_space="Shared")
    h1_slice = nc.dram_tensor("h1_slice", [HSL, HID + 4], dt.float32)
    a2_slice = nc.dram_tensor("a2_slice", [NT1 * P, 2], dt.float32)
    table2 = nc.dram_tensor("table2", [T2R, 2], dt.float32, addr_space="Shared")
    rs_in = nc.dram_tensor("rs_in", [N2 + 8, OUT + 1], dt.float32)
    rs_out = nc.dram_tensor("rs_out", [N2 // C, OUT + 1], dt.float32)

    AF = mybir.ActivationFunctionType
    AL = mybir.AluOpType

    with nc.allow_low_precision(reason="bf16 softmax weights by design"), \
         tile.TileContext(nc) as tc:
        with tc.tile_pool(name="const", bufs=1) as cpool, \
             tc.tile_pool(name="meta", bufs=1) as mpool, \
             tc.tile_pool(name="work", bufs=2) as wpool, \
             tc.tile_pool(name="xarena", bufs=2) as xpool, \
             tc.tile_pool(name="small", bufs=3) as spool, \
             tc.tile_pool(name="ps", bufs=2, space="PSUM") as ps, \
             tc.tile_pool(name="ps1", bufs=1, space="PSUM") as ps1:

            # ---- load constants & metas ----
            def load(pool, src_ap, shape, dtype, tag):
                t = pool.tile(shape, dtype, tag=tag)
                nc.sync.dma_start(t[:], src_ap)
                return t

            t_ident = load(cpool, ident[:], [P, P], dt.float32, tag="t_ident")
            t_identb = cpool.tile([P, P], dt.bfloat16, tag="t_identb")
            nc.vector.tensor_copy(out=t_identb[:], in_=t_ident[:])

            t_irow = load(cpool, iota_row[:], [P, P], dt.bfloat16, tag="t_irow")
            t_ie = load(cpool, iota_e[:], [P, CAP1], dt.float32, tag="t_ie")
            t_wasd = load(cpool, wa_sd1[:], [F, 2*H], dt.float32, tag="t_wasd")
            t_wasdb = cpool.tile([F, 2*H], dt.bfloat16, tag="t_wasdb")
            nc.vector.tensor_copy(out=t_wasdb[:], in_=t_wasd[:])
            t_m1 = load(cpool, m1[:], [F, H*HID], dt.bfloat16, tag="t_m1")
            t_m1w2 = load(cpool, m1w2[:], [F, H*2], dt.bfloat16, tag="t_m1w2")
            t_m2 = load(cpool, m2[:], [P, 2*OUT], dt.bfloat16, tag="t_m2")
            t_c1b = load(cpool, c1_b[:], [P, HID], dt.float32, tag="t_c1b")
            t_c1w2 = load(cpool, c1w2_b[:], [P, 2], dt.float32, tag="t_c1w2")
            t_cc2 = load(cpool, cc2_b[:], [P, OUT], dt.float32, tag="t_cc2")
            t_mxsrc1 = load(mpool, m_xsrc1[:], [P, NB1], dt.int32, tag="t_mxsrc1")
            t_mdl1 = load(mpool, m_dstloc1[:], [P, NB1], dt.float32, tag="t_mdl1")
            t_mst1 = load(mpool, m_start1[:], [P, NT1], dt.float32, tag="t_mst1")
            t_men1 = load(mpool, m_end1[:], [P, NT1], dt.float32, tag="t_men1")
            t_madi1 = load(mpool, m_adidx1[:], [P, NT1], dt.int32, tag="t_madi1")
            t_msrc2 = load(mpool, m_src2[:], [P, NB2], dt.int32, tag="t_msrc2")
            t_mdl2 = load(mpool, m_dstloc2[:], [P, NB2], dt.float32, tag="t_mdl2")
            t_mst2 = load(mpool, m_start2[:], [P, NT2], dt.float32, tag="t_mst2")
            t_men2 = load(mpool, m_end2[:], [P, NT2], dt.float32, tag="t_men2")
            t_madi2 = load(mpool, m_ad2idx[:], [P, NT2], dt.int32, tag="t_madi2")
            t_mo2i = load(mpool, m_o2idx[:], [P, NT2], dt.int32, tag="t_mo2i")

            # warm joins: touch DMA'd tiles so downstream ops need few waits
            warm = spool.tile([P, 1], dt.float32)
            for t_ in (t_ie, t_mdl1, t_mst1, t_men1, t_mdl2, t_mst2, t_men2,
                       t_c1b, t_c1w2, t_cc2):
                nc.vector.tensor_copy(out=warm[:], in_=t_[:, 0:1])
            warmb = spool.tile([P, 1], dt.bfloat16)
            nc.vector.tensor_copy(out=warmb[:], in_=t_irow[:, 0:1])
            nc.vector.tensor_copy(out=warmb[:], in_=t_m1[:, 0:1])
            nc.vector.tensor_copy(out=warmb[:], in_=t_m2[:, 0:1])
            nc.vector.tensor_copy(out=warmb[:], in_=t_m1w2[:, 0:1])

            # ================= dense phase: a_sd1 table slice ================
            sc_dense = nc.enter_named_scope("dense", False)
            for j in range(NSLP // P):
                xt = wpool.tile([P, F], dt.float32, tag="xd")
                nc.sync.dma_start(xt[:], x_slice[j*P:(j+1)*P, :])
                pxT = ps.tile([P, P], dt.float32, space="PSUM", tag="psm1")
                nc.tensor.transpose(out=pxT[:], in_=xt[:], identity=t_ident[:])
                xTs = wpool.tile([P, P], dt.float32, tag="xTs")
                nc.vector.tensor_copy(out=xTs[:], in_=pxT[:])
                pa = ps1.tile([P, 2*H], dt.float32, space="PSUM", tag="psm2")
                nc.tensor.matmul(pa[:], lhsT=xTs[:], rhs=t_wasd[:], start=True, stop=True)
                asb = wpool.tile([P, 2*H], dt.float32, tag="asb")
                nc.vector.tensor_copy(out=asb[:], in_=pa[:])
                nc.sync.dma_start(asd_slice[j*P:(j+1)*P, :], asb[:])

            nc.leave_named_scope("dense", sc_dense[0], False)
            sc_ag1 = nc.enter_named_scope("ag1", False)
            nc.gpsimd.collective_compute(
                "AllGather", AL.bypass, replica_groups=[list(range(C))],
                ins=[asd_slice[:]], outs=[table1[:]])
            nc.leave_named_scope("ag1", sc_ag1[0], False)
            sc_l1 = nc.enter_named_scope("l1", False)

            # ======================= layer-1 edge phase ======================
            for t in range(NT1):
                # tile setup
                adt = spool.tile([P, 2*H], dt.float32, tag="adt")
                nc.gpsimd.indirect_dma_start(
                    out=adt[:], out_offset=None, in_=table1[:],
                    in_offset=bass.IndirectOffsetOnAxis(ap=t_madi1[:, t:t+1], axis=0))
                adt_bf = spool.tile([P, H], dt.bfloat16, tag="adt_bf")
                nc.vector.tensor_copy(out=adt_bf[:], in_=adt[:, H:2*H])
                # selT [128d, CAP1]
                ge_a = wpool.tile([P, CAP1], dt.bfloat16, tag="ge_a")
                ge_b = wpool.tile([P, CAP1], dt.bfloat16, tag="ge_b")
                selT = wpool.tile([P, CAP1], dt.bfloat16, tag="selT")
                nc.vector.tensor_scalar(out=ge_a[:], in0=t_ie[:], scalar1=t_mst1[:, t:t+1],
                                        scalar2=None, op0=AL.is_ge)
                nc.vector.tensor_scalar(out=ge_b[:], in0=t_ie[:], scalar1=t_men1[:, t:t+1],
                                        scalar2=None, op0=AL.is_ge)
                nc.vector.tensor_tensor(out=selT[:], in0=ge_a[:], in1=ge_b[:], op=AL.subtract)

                xar = xpool.tile([P, BLK1 * F], dt.bfloat16, tag="xar")
                sel = wpool.tile([P, BLK1 * P], dt.bfloat16, tag="sel")
                pl = ps.tile([P, BLK1 * H], dt.float32, space="PSUM", tag="psm1")
                for b in range(BLK1):
                    col = t * BLK1 + b
                    # X gather (f32)
                    nc.gpsimd.indirect_dma_start(
                        out=xar[:, b*F:(b+1)*F], out_offset=None, in_=x[:],
                        in_offset=bass.IndirectOffsetOnAxis(ap=t_mxsrc1[:, col:col+1], axis=0))
                    # sel block
                    nc.vector.tensor_scalar(out=sel[:, b*P:(b+1)*P], in0=t_irow[:],
                                            scalar1=t_mdl1[:, col:col+1], scalar2=None,
                                            op0=AL.is_equal)
                    # transpose X block -> XT (for on-chip a_s)
                    pxt = ps.tile([P, P], dt.bfloat16, space="PSUM", tag="psm1", name="pxt")
                    nc.tensor.transpose(out=pxt[:], in_=xar[:, b*F:(b+1)*F],
                                        identity=t_identb[:])
                    xts = spool.tile([P, P], dt.bfloat16, tag="xts")
                    nc.vector.tensor_copy(out=xts[:], in_=pxt[:])
                    # l = a_d (selT expansion) + a_s (XT.T @ Wa_s), one psum group
                    nc.tensor.matmul(pl[:, b*H:(b+1)*H], lhsT=selT[:, b*P:(b+1)*P],
                                     rhs=adt_bf[:], start=True, stop=False)
                    nc.tensor.matmul(pl[:, b*H:(b+1)*H], lhsT=xts[:],
                                     rhs=t_wasdb[:, 0:H], start=False, stop=True)
                # l-chain batched
                lsb = wpool.tile([P, BLK1 * H], dt.float32, tag="lsb")
                nc.vector.tensor_copy(out=lsb[:], in_=pl[:])
                lr = wpool.tile([P, BLK1 * H], dt.float32, tag="lr")
                nc.vector.scalar_tensor_tensor(out=lr[:], in0=lsb[:], scalar=NEG,
                                               in1=lsb[:], op0=AL.mult, op1=AL.max)
                pexp = wpool.tile([P, BLK1 * H], dt.bfloat16, tag="pexp")
                nc.scalar.activation(pexp[:], lr[:], AF.Exp)
                # denominators
                pd = ps1.tile([P, H], dt.float32, space="PSUM", tag="psm2")
                for b in range(BLK1):
                    nc.tensor.matmul(pd[:], lhsT=sel[:, b*P:(b+1)*P],
                                     rhs=pexp[:, b*H:(b+1)*H],
                                     start=(b == 0), stop=(b == BLK1 - 1))
                dsb = spool.tile([P, H], dt.float32, tag="dsb")
                nc.scalar.activation(dsb[:], pd[:], AF.Copy, bias=1e-16)
                rec = spool.tile([P, H], dt.bfloat16, tag="rec")
                nc.vector.reciprocal(out=rec[:], in_=dsb[:])
                # alpha-hat = p * (selT.T @ rec)
                pr = ps.tile([P, BLK1 * H], dt.float32, space="PSUM", tag="psm1")
                for b in range(BLK1):
                    nc.tensor.matmul(pr[:, b*H:(b+1)*H], lhsT=selT[:, b*P:(b+1)*P],
                                     rhs=rec[:], start=True, stop=True)
                alph = wpool.tile([P, BLK1 * H], dt.float32, tag="alph")
                nc.vector.tensor_tensor(out=alph[:], in0=pexp[:], in1=pr[:], op=AL.mult)
                # aggregation (transposed): psum_aggT [128f, 4*128d]
                paggs = [ps1.tile([P, P], dt.float32, space="PSUM", tag=f"pagg{h}", name=f"paggs{h}")
                         for h in range(H)]
                for b in range(BLK1):
                    msg = spool.tile([P, H * F], dt.bfloat16, tag="msg")
                    for h in range(H - 1):
                        nc.vector.tensor_scalar(
                            out=msg[:, h*F:(h+1)*F], in0=xar[:, b*F:(b+1)*F],
                            scalar1=alph[:, b*H+h:b*H+h+1], scalar2=None, op0=AL.mult)
                    nc.scalar.activation(
                        msg[:, (H-1)*F:H*F], xar[:, b*F:(b+1)*F], AF.Copy,
                        scale=alph[:, b*H+H-1:b*H+H])
                    for h in range(H):
                        nc.tensor.matmul(paggs[h][:], lhsT=msg[:, h*F:(h+1)*F],
                                         rhs=sel[:, b*P:(b+1)*P],
                                         start=(b == 0), stop=(b == BLK1 - 1))
                aggT = wpool.tile([P, H * P], dt.bfloat16, tag="aggT")
                for h in range(H):
                    nc.vector.tensor_copy(out=aggT[:, h*P:(h+1)*P], in_=paggs[h][:])
                # h1 = sum_h aggT_h.T @ M1_h + c1 ;  a2 = sum_h aggT_h.T @ M1W2_h + c1W2
                ph1 = ps1.tile([P, HID], dt.float32, space="PSUM", tag="pmed")
                pa2 = ps1.tile([P, 2], dt.float32, space="PSUM", tag="psm2")
                for h in range(H):
                    nc.tensor.matmul(ph1[:], lhsT=aggT[:, h*P:(h+1)*P],
                                     rhs=t_m1[:, h*HID:(h+1)*HID],
                                     start=(h == 0), stop=(h == H - 1))
                    nc.tensor.matmul(pa2[:], lhsT=aggT[:, h*P:(h+1)*P],
                                     rhs=t_m1w2[:, h*2:(h+1)*2],
                                     start=(h == 0), stop=(h == H - 1))
                if DEBUG and t == 0:
                    nc.sync.dma_start(dbg_selT[:], selT[:])
                    nc.sync.dma_start(dbg_sel[:], sel[:])
                    nc.sync.dma_start(dbg_l[:], lr[:])
                    nc.sync.dma_start(dbg_p[:], pexp[:])
                    nc.sync.dma_start(dbg_rec[:], rec[:])
                    nc.sync.dma_start(dbg_al[:], alph[:])
                    nc.sync.dma_start(dbg_aggT[:], aggT[:])
                    nc.sync.dma_start(dbg_adt[:], adt[:])
                    nc.sync.dma_start(dbg_xar[:], xar[:])
                    nc.sync.dma_start(dbg_asg[:, 0:BLK1*H], lsb[:])
                h1sb = wpool.tile([P, HID + 4], dt.float32, tag="h1sb")
                nc.vector.tensor_tensor(out=h1sb[:, 0:HID], in0=ph1[:], in1=t_c1b[:], op=AL.add)
                if DEBUG and t == 0:
                    nc.sync.dma_start(dbg_h1[:], h1sb[:])
                a2sb = spool.tile([P, 2], dt.float32, tag="a2sb")
                nc.vector.tensor_tensor(out=a2sb[:], in0=pa2[:], in1=t_c1w2[:], op=AL.add)
                nc.vector.tensor_copy(out=h1sb[:, HID:HID+1], in_=a2sb[:, 0:1])
                nc.sync.dma_start(h1_slice[t*P:(t+1)*P, :], h1sb[:])
                nc.sync.dma_start(a2_slice[t*P:(t+1)*P, :], a2sb[:])
                if DEBUG and t == 0:
                    nc.sync.dma_start(dbg_a2[:], a2sb[:])

            nc.leave_named_scope("l1", sc_l1[0], False)
            sc_ag2 = nc.enter_named_scope("ag2", False)
            nc.gpsimd.collective_compute(
                "AllGather", AL.bypass, replica_groups=[list(range(C))],
                ins=[a2_slice[:]], outs=[table2[:]])
            nc.leave_named_scope("ag2", sc_ag2[0], False)
            sc_l2 = nc.enter_named_scope("l2", False)

            # ======================= layer-2 edge phase ======================
            for t in range(NT2):
                ad2 = spool.tile([P, 2], dt.float32, tag="ad2")
                nc.gpsimd.indirect_dma_start(
                    out=ad2[:], out_offset=None, in_=table2[:],
                    in_offset=bass.IndirectOffsetOnAxis(ap=t_madi2[:, t:t+1], axis=0))
                ad2_bf = spool.tile([P, 1], dt.bfloat16, tag="ad2_bf")
                nc.vector.tensor_copy(out=ad2_bf[:], in_=ad2[:, 1:2])
                ge_a2 = wpool.tile([P, CAP2], dt.bfloat16, tag="ge_a2")
                ge_b2 = wpool.tile([P, CAP2], dt.bfloat16, tag="ge_b2")
                selT2 = wpool.tile([P, CAP2], dt.bfloat16, tag="selT2")
                nc.vector.tensor_scalar(out=ge_a2[:], in0=t_ie[:, 0:CAP2],
                                        scalar1=t_mst2[:, t:t+1], scalar2=None, op0=AL.is_ge)
                nc.vector.tensor_scalar(out=ge_b2[:], in0=t_ie[:, 0:CAP2],
                                        scalar1=t_men2[:, t:t+1], scalar2=None, op0=AL.is_ge)
                nc.vector.tensor_tensor(out=selT2[:], in0=ge_a2[:], in1=ge_b2[:], op=AL.subtract)

                HW2 = HID + 4
                h1g = xpool.tile([P, BLK2 * HW2], dt.float32, tag="h1g")
                h1gb = xpool.tile([P, BLK2 * HID], dt.bfloat16, tag="h1gb")
                sel2 = wpool.tile([P, BLK2 * P], dt.bfloat16, tag="sel2")
                pl2 = ps.tile([P, BLK2], dt.float32, space="PSUM", tag="psm1")
                for b in range(BLK2):
                    col = t * BLK2 + b
                    nc.gpsimd.indirect_dma_start(
                        out=h1g[:, b*HW2:(b+1)*HW2], out_offset=None, in_=h1_slice[:],
                        in_offset=bass.IndirectOffsetOnAxis(ap=t_msrc2[:, col:col+1], axis=0))
                    nc.vector.tensor_scalar(out=sel2[:, b*P:(b+1)*P], in0=t_irow[:],
                                            scalar1=t_mdl2[:, col:col+1], scalar2=None,
                                            op0=AL.is_equal)
                    nc.tensor.matmul(pl2[:, b:b+1], lhsT=selT2[:, b*P:(b+1)*P],
                                     rhs=ad2_bf[:], start=True, stop=True)
                nc.vector.tensor_copy(
                    out=h1gb[:].rearrange("p (b k) -> p b k", k=HID),
                    in_=h1g[:].rearrange("p (b k) -> p b k", k=HW2)[:, :, 0:HID])
                l2sb = spool.tile([P, BLK2], dt.float32, tag="l2sb")
                nc.vector.tensor_tensor(
                    out=l2sb[:].rearrange("p (b k) -> p b k", k=1),
                    in0=pl2[:].rearrange("p (b k) -> p b k", k=1),
                    in1=h1g[:].rearrange("p (b k) -> p b k", k=HW2)[:, :, HID:HID+1],
                    op=AL.add)
                lr2 = spool.tile([P, BLK2], dt.float32, tag="lr2")
                nc.vector.scalar_tensor_tensor(out=lr2[:], in0=l2sb[:], scalar=NEG,
                                               in1=l2sb[:], op0=AL.mult, op1=AL.max)
                pexp2 = spool.tile([P, BLK2], dt.float32, tag="pexp2")
                nc.scalar.activation(pexp2[:], lr2[:], AF.Exp)
                pexp2b = spool.tile([P, BLK2], dt.bfloat16, tag="pexp2b")
                nc.vector.tensor_copy(out=pexp2b[:], in_=pexp2[:])
                # unnormalized aggregation + denominators
                pd2 = ps1.tile([P, 1], dt.float32, space="PSUM", tag="psm2")
                pagg2s = [ps1.tile([P, P], dt.float32, space="PSUM", tag=f"pagg{kk}", name=f"pagg2s{kk}")
                          for kk in range(2)]
                for b in range(BLK2):
                    nc.tensor.matmul(pd2[:], lhsT=sel2[:, b*P:(b+1)*P],
                                     rhs=pexp2b[:, b:b+1],
                                     start=(b == 0), stop=(b == BLK2 - 1))
                    msg2 = spool.tile([P, HID], dt.bfloat16, tag="msg2")
                    nc.vector.tensor_scalar(out=msg2[:], in0=h1gb[:, b*HID:(b+1)*HID],
                                            scalar1=pexp2[:, b:b+1], scalar2=None, op0=AL.mult)
                    for kk in range(2):
                        nc.tensor.matmul(pagg2s[kk][:], lhsT=msg2[:, kk*P:(kk+1)*P],
                                         rhs=sel2[:, b*P:(b+1)*P],
                                         start=(b == 0), stop=(b == BLK2 - 1))
                aggT2 = wpool.tile([P, 2 * P], dt.bfloat16, tag="aggT2")
                for kk in range(2):
                    nc.vector.tensor_copy(out=aggT2[:, kk*P:(kk+1)*P], in_=pagg2s[kk][:])
                po2 = ps1.tile([P, OUT], dt.float32, space="PSUM", tag="pmed")
                for k in range(2):
                    nc.tensor.matmul(po2[:], lhsT=aggT2[:, k*P:(k+1)*P],
                                     rhs=t_m2[:, k*OUT:(k+1)*OUT],
                                     start=(k == 0), stop=(k == 1))
                o2sb = wpool.tile([P, OUT + 1], dt.float32, tag="o2sb")
                nc.vector.tensor_copy(out=o2sb[:, 0:OUT], in_=po2[:])
                nc.vector.tensor_copy(out=o2sb[:, OUT:OUT+1], in_=pd2[:])
                nc.gpsimd.indirect_dma_start(
                    out=rs_in[:], out_offset=bass.IndirectOffsetOnAxis(ap=t_mo2i[:, t:t+1], axis=0),
                    in_=o2sb[:], in_offset=None)

            nc.leave_named_scope("l2", sc_l2[0], False)
            sc_rs = nc.enter_named_scope("rs", False)
            nc.gpsimd.collective_compute(
                "ReduceScatter", AL.add, replica_groups=[list(range(C))],
                ins=[rs_in[0:N2, :]], outs=[rs_out[:]])
            nc.leave_named_scope("rs", sc_rs[0], False)
            sc_fin = nc.enter_named_scope("fin", False)

            # ---- finalize: divide by denom, add bias ----
            NOUT = N2 // C
            for j in range(math.ceil(NOUT / P)):
                r0 = j * P
                nr = min(P, NOUT - r0)
                fin = wpool.tile([P, OUT + 1], dt.float32, tag="fin")
                nc.sync.dma_start(fin[:nr, :], rs_out[r0:r0+nr, :])
                recf = spool.tile([P, 1], dt.float32, tag="recf")
                nc.vector.reciprocal(out=recf[:nr], in_=fin[:nr, OUT:OUT+1])
                osb = wpool.tile([P, OUT], dt.float32, tag="osb")
                nc.vector.tensor_scalar(out=osb[:nr], in0=fin[:nr, 0:OUT],
                                        scalar1=recf[:nr], scalar2=None, op0=AL.mult)
                nc.vector.tensor_tensor(out=osb[:nr], in0=osb[:nr], in1=t_cc2[:nr],
                                        op=AL.add)
                nc.sync.dma_start(out_ext[r0:r0+nr, :], osb[:nr])
            nc.leave_named_scope("fin", sc_fin[0], False)

    nc.compile()
    return nc


_CACHE = {}


def kernel(**inputs) -> np.ndarray:
    in_maps, plan = preprocess(inputs)
    key = (plan["NT1"], plan["NT2"])
    if key not in _CACHE:
        _CACHE[key] = build_kernel(*key)
    nc = _CACHE[key]
    res = run_bass_kernel_spmd(nc, in_maps, list(range(C))).results
    out = np.concatenate([res[c]["out"] for c in range(C)], axis=0)
    return out.astype(np.float32)

